# revision 43
# baseline (speedup 1.0000x reference)
"""Trainium2 Bass kernel for AdaptiveFocusedLoss, data-parallel over 8 NeuronCores.

Math (matches the jax reference exactly, up to float rounding):
  logp = log_softmax(outputs); base = -mean(logp[i, l_i])
  probs = softmax(outputs); w = W[l_i]
  mask = (c != l_i) & (w > 1) & (p > 0.2)
  penalty = sum(w*p*mask) / max(count,1) if count>0 else 0
  loss = base + 0.5 * penalty

Device-side pipeline (per core, rows sharded; group layout [p, t, c] with
t = chunk index (ch per group), c innermost so matmul chunks are contiguous):
  e = exp(x)            (ACT, bf16; x = 5*randn bounded ~±30, safe without max-sub)
  s[p,t] = sum_c e      (DVE: one strided f32 tensor_reduce per group)
  r = 1/s               (DVE reciprocal, f32)
  p = e*r               (chunk 0 on DVE tensor_tensor; chunks 1..ch on GPSIMD
                         with a 3D stride-0 broadcast AP for r)
  A  = [p > 0.2]        (DVE tensor_scalar is_gt, 4x mode) -> rhs region 1
  M2 = max(p-0.2, 0)    (chunks [0,T_M2_DVE): DVE dual-op ts 4x; rest: ACT Relu)
                        -> rhs region 0
  region 2 = x (bf16, straight from DMA); region 3 = onehot (bf16, from DMA)
  PSUM accumulates over all 128-row chunks:
     S_M2 += O^T @ M2 ; T += O^T @ A ; R += O^T @ x
  epilogue: lnz_sum[p] = sum_t ln(s_all[p,t])
The software pipeline runs three stages (head: DMA+exp; tailA: rowsum+recip+
p-mult; tailB: mask ops+matmuls) at depths g, g-4, g-6 so the long GPSIMD
multiply of one group overlaps DVE/ACT work of neighboring groups.
Host side:
  ce_sum  = sum(lnz) - trace(R)            (trace(R) = sum_i x[i, l_i])
  pen_sum = <G0, S_M2 + 0.2*T>,  count = <H0, T>
  where G0 = W*(W>1) diag-zeroed, H0 = (W>1) diag-zeroed  (c != l mask == zero diag)
"""

import numpy as np

try:
    from concourse import bass, mybir, tile
    from concourse.bass_utils import run_bass_kernel_spmd
except ImportError:  # pragma: no cover
    import sys

    sys.path.insert(0, "/opt/trn_rl_repo")
    from concourse import bass, mybir, tile
    from concourse.bass_utils import run_bass_kernel_spmd

F32 = mybir.dt.float32
BF16 = mybir.dt.bfloat16
AF = mybir.ActivationFunctionType
OP = mybir.AluOpType
AX = mybir.AxisListType

N_CORES = 8
C = 128  # num classes
B_FULL = 524288
PROB_THRESH = 0.2
CONF_PEN = 0.5
WEIGHT_THRESH = 1.0

GROUP_ROWS = 2048  # rows per group (ch = 16 chunks)

# Engine-balance splits along the chunk (t) axis, out of ch chunks/group:
# p = e*r: chunks [0, T_P_DVE) on DVE (3D broadcast tensor_tensor), rest GPSIMD.
# M2:      chunks [0, T_M2_DVE) on DVE (dual-op ts, immediate scalars), rest ACT.
# NOTE: tensor_scalar with an AP scalar (TensorScalarPtr) measured ~28ns/elem
# on HW — never use it for bulk work; immediate-scalar ts is 4x-fast.
T_P_DVE = 0
T_M2_DVE = 0


def build_bass(rows: int, group_rows: int = GROUP_ROWS) -> "bass.Bass":
    """One NeuronCore's graph; SPMD across cores with different shards."""
    assert rows % group_rows == 0 and group_rows % C == 0
    ch = group_rows // C  # chunks (of 128 rows) per group
    ng = rows // group_rows  # groups
    nchunk = rows // C  # total 128-row chunks
    FD = group_rows  # free dim of the big tiles

    nc = bass.Bass()
    # xoh[p, g, 0, t, c] = x_bf16[row(g,p,t), c]; xoh[p, g, 1, t, c] = onehot.
    # One DMA per group loads both with 2*ch*C*2 = 8KB contiguous runs per
    # partition (128 big descriptors).
    xoh_ext = nc.declare_dram_parameter("xoh", [C, ng * 2 * FD], BF16, isOutput=False)
    out_ext = nc.declare_dram_parameter("out", [C, 6 * C + 1], F32, isOutput=True)
    xoh_view = xoh_ext[:, :].rearrange("p (g u f) -> p g u f", g=ng, u=2)

    with tile.TileContext(nc) as tc:
        with (
            tc.tile_pool(name="const", bufs=1) as constp,
            tc.tile_pool(name="ebuf", bufs=6) as ep,
            tc.tile_pool(name="pbuf", bufs=10) as pp,
            tc.tile_pool(name="rhsbuf", bufs=8) as rhsp,
            tc.tile_pool(name="small", bufs=8) as smallp,
            tc.tile_pool(name="psum", bufs=1, space="PSUM") as psp,
        ):
            s_all = constp.tile([C, nchunk], F32)
            ln_t = constp.tile([C, nchunk], F32)
            out_sb = constp.tile([C, 6 * C + 1], F32)
            nthr = constp.tile([C, 1], F32)  # -PROB_THRESH bias for ACT Relu
            acc = psp.tile([C, 3 * C], F32)
            nc.vector.memset(nthr[:], -PROB_THRESH)

            state = {}

            def head(g):
                """DMA + exp for group g (emitted ahead of tail).
                rhs regions: [M2(FD) | A(FD) | X(FD) | OH(FD)]."""
                et = ep.tile([C, FD], BF16, tag="et")
                rhs = rhsp.tile([C, 4 * FD], BF16, tag="rhs")
                rhs4 = rhs[:].rearrange("p (u f) -> p u f", u=4)
                nc.sync.dma_start(rhs4[:, 2:4, :], xoh_view[:, g, :, :])
                nc.scalar.activation(et[:], rhs[:, 2 * FD : 3 * FD], AF.Exp)
                state[g] = (et, rhs)

            def tailA(g):
                """Rowsum + recip + DVE share of p, and kick off the GPSIMD
                p-multiply. Emitted one group ahead of tailB so DVE/ACT work
                of group g overlaps GPSIMD's long multiply of group g."""
                et, rhs = state.pop(g)
                pt = pp.tile([C, FD], BF16, tag="pt")
                rt = smallp.tile([C, ch], F32, tag="rt")

                # rowsum: one strided f32 reduce per group (fewer DVE ops beats
                # the bf16 halving tree once per-op stall overhead is counted)
                e3 = et[:].rearrange("p (t c) -> p t c", t=ch)
                ssl = s_all[:, g * ch : (g + 1) * ch]
                nc.vector.reduce_sum(out=ssl, in_=e3[:], axis=AX.X)
                nc.vector.reciprocal(rt[:], ssl)

                # p = e * r (DVE chunk share + GPSIMD broadcast mult)
                pt3 = pt[:].rearrange("p (t c) -> p t c", t=ch)
                cs = T_P_DVE
                rtb = rt[:].rearrange("p (t x) -> p t x", x=1)
                with nc.allow_low_precision(reason="bf16 p"):
                    if cs > 0:
                        nc.vector.tensor_tensor(
                            pt3[:, :cs, :],
                            e3[:, :cs, :],
                            rtb[:, :cs, :].to_broadcast([C, cs, C]),
                            OP.mult,
                        )
                    nc.gpsimd.tensor_tensor(
                        pt3[:, cs:, :],
                        e3[:, cs:, :],
                        rtb[:, cs:, :].to_broadcast([C, ch - cs, C]),
                        OP.mult,
                    )
                state[("b", g)] = (rhs, pt)

            def tailB(g):
                """Mask ops (need the full p) + matmuls for group g."""
                rhs, pt = state.pop(("b", g))
                # A = [p > 0.2] -> region 1 (DVE tensor_scalar, 4x mode)
                nc.vector.tensor_scalar(
                    rhs[:, FD : 2 * FD], pt[:], PROB_THRESH, None, OP.is_gt
                )
                # M2 = max(p - 0.2, 0) -> region 0 (DVE dual-op ts | ACT Relu)
                ms = T_M2_DVE * C
                if ms > 0:
                    nc.vector.tensor_scalar(
                        rhs[:, 0:ms], pt[:, 0:ms], PROB_THRESH, 0.0, OP.subtract, OP.max
                    )
                nc.scalar.activation(
                    rhs[:, ms:FD], pt[:, ms:FD], AF.Relu, bias=nthr[:, 0:1]
                )

                # scatter-accumulate into PSUM: [S_M2 | T | R]
                rhs5 = rhs[:].rearrange("p (u t c) -> p u t c", u=4, c=C)
                for j in range(ch):
                    first = g == 0 and j == 0
                    last = g == ng - 1 and j == ch - 1
                    nc.tensor.matmul(
                        acc[:, :],
                        rhs5[:, 3, j, :],
                        rhs5[:, 0:3, j, :],
                        start=first,
                        stop=last,
                    )

            da = min(4, ng)  # head runs `da` groups ahead of tailA
            db = min(da + 4, ng)  # tailB two more groups behind (GPS slack)
            for g in range(ng):
                head(g)
                if g >= da:
                    tailA(g - da)
                if g >= db:
                    tailB(g - db)
            for g in range(ng - da, ng):
                tailA(g)
            for g in range(ng - db, ng):
                tailB(g)

            # epilogue: sum of log-partition-functions, dump accumulators
            nc.scalar.activation(ln_t[:], s_all[:], AF.Ln)
            nc.vector.reduce_sum(
                out=out_sb[:, 6 * C : 6 * C + 1], in_=ln_t[:], axis=AX.X, op=OP.add
            )
            nc.vector.tensor_copy(out_sb[:, 0 : 3 * C], acc[:, :])
            nc.vector.memset(out_sb[:, 3 * C : 6 * C], 0.0)
            nc.sync.dma_start(out_ext[:, :], out_sb[:])

    _strip_redundant_dma_lane_waits(nc)
    return nc


def _strip_redundant_dma_lane_waits(nc):
    """Every TPB instruction encoding holds exactly ONE sync-wait slot; walrus
    raises "Too many sync wait commands" on the rest. Legalize every
    multi-wait instruction: keep ONE wait embedded, hoist the rest into
    standalone InstEventSemaphore waits on the same queue immediately before
    the instruction.

    For DMAs the EMBEDDED wait must be the DMA-lane predecessor wait when one
    exists: it enforces in-order completion within the lane, which the
    cumulative semaphore thresholds consumers wait on REQUIRE for soundness
    (out-of-order completion would satisfy a threshold before the data
    landed). Engine waits are hoisted onto the issuing sequencer queue, which
    executes them before pushing the DMA to the ring."""
    f = nc.m.functions[0]
    for blk in list(f.blocks):
        insts = list(blk.instructions)
        new_insts = []
        changed = False
        for inst in insts:
            si = inst.sync_info
            waits = list(si.on_wait) if (si and si.on_wait) else []
            if len(waits) > 1:
                changed = True
                if type(inst).__name__ == "InstDMACopy":
                    lane = [
                        w for w in waits if w.ant_name.startswith(("DMAHW", "DMASW"))
                    ]
                    eng = [
                        w
                        for w in waits
                        if not w.ant_name.startswith(("DMAHW", "DMASW"))
                    ]
                    assert len(lane) <= 1, f"{inst.name}: {len(lane)} lane waits"
                    keep = lane if lane else eng[-1:]
                    extra = eng if lane else eng[:-1]
                else:
                    keep = waits[-1:]
                    extra = waits[:-1]
                for k, w in enumerate(extra):
                    es = mybir.InstEventSemaphore(
                        name=f"{inst.name}-wsplit{k}",
                        engine=inst.engine,
                        ins=[],
                        outs=[],
                        sync_info=mybir.SyncInfo(on_wait=[w], on_update=[]),
                    )
                    nc.register_instruction(es)
                    new_insts.append(es)
                si.on_wait = keep
            new_insts.append(inst)
        if changed:
            blk.instructions = new_insts


def _shard_inputs(outputs: np.ndarray, labels: np.ndarray, rows: int, group_rows: int):
    """Build per-core in_maps. Row mapping inside a core/group: row = g*G + p*ch + t."""
    import ml_dtypes

    bf16 = ml_dtypes.bfloat16
    ch = group_rows // C
    ng = rows // group_rows
    in_maps = []
    n_cores = outputs.shape[0] // rows
    cls = np.arange(C, dtype=np.int32)
    for i in range(n_cores):
        lab_i = labels[i * rows : (i + 1) * rows].astype(np.int32)
        labT = lab_i.reshape(ng, C, ch).transpose(1, 0, 2)  # [C, ng, ch]
        oh = labT[:, :, :, None] == cls[None, None, None, :]  # [C, ng, ch, C]
        xb = (
            outputs[i * rows : (i + 1) * rows]
            .astype(bf16)
            .reshape(ng, C, ch, C)
            .transpose(1, 0, 2, 3)
        )  # [C, ng, ch, C]
        xoh = np.stack([xb, oh.astype(bf16)], axis=2)  # [C, ng, 2, ch, C]
        in_maps.append({"xoh": np.ascontiguousarray(xoh.reshape(C, ng * 2 * group_rows))})
    return in_maps


def combine_outputs(core_outs, lnz_extra=None, confusion_weights=None, B=None):
    """Host-side reduction of per-core [128, 769] partials -> scalar loss."""
    S_M2 = np.zeros((C, C), np.float64)
    T = np.zeros((C, C), np.float64)
    R = np.zeros((C, C), np.float64)
    lnz_sum = 0.0
    for o in core_outs:
        o = np.asarray(o, np.float64)
        for base in (0, 3 * C):
            S_M2 += o[:, base : base + C]
            T += o[:, base + C : base + 2 * C]
            R += o[:, base + 2 * C : base + 3 * C]
        lnz_sum += o[:, 6 * C].sum()
    ce_sum = lnz_sum - np.trace(R)
    base_loss = ce_sum / B

    W = np.asarray(confusion_weights, np.float64)
    wmask = W > WEIGHT_THRESH
    G0 = np.where(wmask, W, 0.0)
    np.fill_diagonal(G0, 0.0)
    H0 = wmask.astype(np.float64)
    np.fill_diagonal(H0, 0.0)

    S = S_M2 + PROB_THRESH * T
    pen_sum = float((G0 * S).sum())
    count = float(np.rint((H0 * T).sum()))
    penalty = pen_sum / max(count, 1.0) if count > 0 else 0.0
    return np.float32(base_loss + CONF_PEN * penalty)


_CACHE = {}


def _get_nc(rows: int, group_rows: int):
    key = (rows, group_rows)
    if key not in _CACHE:
        _CACHE[key] = build_bass(rows, group_rows)
    return _CACHE[key]


def kernel(outputs: np.ndarray, labels: np.ndarray, confusion_weights: np.ndarray, **kw):
    outputs = np.asarray(outputs, np.float32)
    labels = np.asarray(labels)
    B = outputs.shape[0]
    rows = B // N_CORES
    group_rows = GROUP_ROWS
    nc = _get_nc(rows, group_rows)
    in_maps = _shard_inputs(outputs, labels, rows, group_rows)
    res = run_bass_kernel_spmd(nc, in_maps, core_ids=list(range(N_CORES)))
    core_outs = [r["out"] for r in res.results]
    return combine_outputs(core_outs, confusion_weights=confusion_weights, B=B)


if __name__ == "__main__":
    # smoke test on random data (host-side check only builds the graph)
    nc = build_bass(8192, GROUP_ROWS)
    print("built ok:", nc)


# revision 44
# speedup vs baseline: 1.1775x; 1.1775x over previous
"""Trainium2 Bass kernel for AdaptiveFocusedLoss, data-parallel over 8 NeuronCores.

Math (matches the jax reference exactly, up to float rounding):
  logp = log_softmax(outputs); base = -mean(logp[i, l_i])
  probs = softmax(outputs); w = W[l_i]
  mask = (c != l_i) & (w > 1) & (p > 0.2)
  penalty = sum(w*p*mask) / max(count,1) if count>0 else 0
  loss = base + 0.5 * penalty

Device-side pipeline (per core, rows sharded; group layout [p, t, c] with
t = chunk index (ch per group), c innermost so matmul chunks are contiguous):
  e = exp(x)            (ACT, bf16; x = 5*randn bounded ~±30, safe without max-sub)
  s[p,t] = sum_c e      (DVE: one strided f32 tensor_reduce per group)
  r = 1/s               (DVE reciprocal, f32)
  p = e*r               (chunk 0 on DVE tensor_tensor; chunks 1..ch on GPSIMD
                         with a 3D stride-0 broadcast AP for r)
  A  = [p > 0.2]        (DVE tensor_scalar is_gt, 4x mode) -> rhs region 1
  M2 = max(p-0.2, 0)    (chunks [0,T_M2_DVE): DVE dual-op ts 4x; rest: ACT Relu)
                        -> rhs region 0
  region 2 = x (bf16, straight from DMA); region 3 = onehot (bf16, from DMA)
  PSUM accumulates over all 128-row chunks:
     S_M2 += O^T @ M2 ; T += O^T @ A ; R += O^T @ x
  epilogue: lnz_sum[p] = sum_t ln(s_all[p,t])
The software pipeline runs three stages (head: DMA+exp; tailA: rowsum+recip+
p-mult; tailB: mask ops+matmuls) at depths g, g-4, g-6 so the long GPSIMD
multiply of one group overlaps DVE/ACT work of neighboring groups.
Host side:
  ce_sum  = sum(lnz) - trace(R)            (trace(R) = sum_i x[i, l_i])
  pen_sum = <G0, S_M2 + 0.2*T>,  count = <H0, T>
  where G0 = W*(W>1) diag-zeroed, H0 = (W>1) diag-zeroed  (c != l mask == zero diag)
"""

import numpy as np

try:
    from concourse import bass, mybir, tile
    from concourse.bass_utils import run_bass_kernel_spmd
except ImportError:  # pragma: no cover
    import sys

    sys.path.insert(0, "/opt/trn_rl_repo")
    from concourse import bass, mybir, tile
    from concourse.bass_utils import run_bass_kernel_spmd

F32 = mybir.dt.float32
BF16 = mybir.dt.bfloat16
AF = mybir.ActivationFunctionType
OP = mybir.AluOpType
AX = mybir.AxisListType

N_CORES = 8
C = 128  # num classes
B_FULL = 524288
PROB_THRESH = 0.2
CONF_PEN = 0.5
WEIGHT_THRESH = 1.0

GROUP_ROWS = 2048  # rows per group (ch = 16 chunks)

# Engine-balance splits along the chunk (t) axis, out of ch chunks/group:
# p = e*r: chunks [0, T_P_DVE) on DVE (3D broadcast tensor_tensor), rest GPSIMD.
# M2:      chunks [0, T_M2_DVE) on DVE (dual-op ts, immediate scalars), rest ACT.
# NOTE: tensor_scalar with an AP scalar (TensorScalarPtr) measured ~28ns/elem
# on HW — never use it for bulk work; immediate-scalar ts is 4x-fast.
T_P_DVE = 0
T_M2_DVE = 0


def build_bass(rows: int, group_rows: int = GROUP_ROWS) -> "bass.Bass":
    """One NeuronCore's graph; SPMD across cores with different shards."""
    assert rows % group_rows == 0 and group_rows % C == 0
    ch = group_rows // C  # chunks (of 128 rows) per group
    ng = rows // group_rows  # groups
    nchunk = rows // C  # total 128-row chunks
    FD = group_rows  # free dim of the big tiles

    nc = bass.Bass()
    # xoh[p, g, 0, t, c] = x_bf16[row(g,p,t), c]; xoh[p, g, 1, t, c] = onehot.
    # One DMA per group loads both with 2*ch*C*2 = 8KB contiguous runs per
    # partition (128 big descriptors).
    xoh_ext = nc.declare_dram_parameter("xoh", [C, ng * 2 * FD], BF16, isOutput=False)
    out_ext = nc.declare_dram_parameter("out", [C, 6 * C + 1], F32, isOutput=True)
    xoh_view = xoh_ext[:, :].rearrange("p (g u f) -> p g u f", g=ng, u=2)

    with tile.TileContext(nc) as tc:
        with (
            tc.tile_pool(name="const", bufs=1) as constp,
            tc.tile_pool(name="ebuf", bufs=6) as ep,
            tc.tile_pool(name="pbuf", bufs=10) as pp,
            tc.tile_pool(name="rhsbuf", bufs=8) as rhsp,
            tc.tile_pool(name="small", bufs=8) as smallp,
            tc.tile_pool(name="psum", bufs=1, space="PSUM") as psp,
        ):
            s_all = constp.tile([C, nchunk], F32)
            ln_t = constp.tile([C, nchunk], F32)
            out_sb = constp.tile([C, 6 * C + 1], F32)
            nthr = constp.tile([C, 1], F32)  # -PROB_THRESH bias for ACT Relu
            acc = psp.tile([C, 3 * C], F32)
            nc.vector.memset(nthr[:], -PROB_THRESH)

            state = {}

            def head(g):
                """DMA + exp for group g (emitted ahead of tail).
                rhs regions: [M2(FD) | A(FD) | X(FD) | OH(FD)]."""
                et = ep.tile([C, FD], BF16, tag="et")
                rhs = rhsp.tile([C, 4 * FD], BF16, tag="rhs")
                rhs4 = rhs[:].rearrange("p (u f) -> p u f", u=4)
                nc.sync.dma_start(rhs4[:, 2:4, :], xoh_view[:, g, :, :])
                nc.scalar.activation(et[:], rhs[:, 2 * FD : 3 * FD], AF.Exp)
                state[g] = (et, rhs)

            def tailA(g):
                """Rowsum + recip + DVE share of p, and kick off the GPSIMD
                p-multiply. Emitted one group ahead of tailB so DVE/ACT work
                of group g overlaps GPSIMD's long multiply of group g."""
                et, rhs = state.pop(g)
                pt = pp.tile([C, FD], BF16, tag="pt")
                rt = smallp.tile([C, ch], F32, tag="rt")

                # rowsum: one strided f32 reduce per group (fewer DVE ops beats
                # the bf16 halving tree once per-op stall overhead is counted)
                e3 = et[:].rearrange("p (t c) -> p t c", t=ch)
                ssl = s_all[:, g * ch : (g + 1) * ch]
                nc.vector.reduce_sum(out=ssl, in_=e3[:], axis=AX.X)
                nc.vector.reciprocal(rt[:], ssl)

                # p = e * r (DVE chunk share + GPSIMD broadcast mult)
                pt3 = pt[:].rearrange("p (t c) -> p t c", t=ch)
                cs = T_P_DVE
                rtb = rt[:].rearrange("p (t x) -> p t x", x=1)
                with nc.allow_low_precision(reason="bf16 p"):
                    if cs > 0:
                        nc.vector.tensor_tensor(
                            pt3[:, :cs, :],
                            e3[:, :cs, :],
                            rtb[:, :cs, :].to_broadcast([C, cs, C]),
                            OP.mult,
                        )
                    nc.gpsimd.tensor_tensor(
                        pt3[:, cs:, :],
                        e3[:, cs:, :],
                        rtb[:, cs:, :].to_broadcast([C, ch - cs, C]),
                        OP.mult,
                    )
                state[("b", g)] = (rhs, pt)

            def tailB(g):
                """Mask ops (need the full p) + matmuls for group g."""
                rhs, pt = state.pop(("b", g))
                # A = [p > 0.2] -> region 1 (DVE tensor_scalar, 4x mode)
                nc.vector.tensor_scalar(
                    rhs[:, FD : 2 * FD], pt[:], PROB_THRESH, None, OP.is_gt
                )
                # M2 = max(p - 0.2, 0) -> region 0 (DVE dual-op ts | ACT Relu)
                ms = T_M2_DVE * C
                if ms > 0:
                    nc.vector.tensor_scalar(
                        rhs[:, 0:ms], pt[:, 0:ms], PROB_THRESH, 0.0, OP.subtract, OP.max
                    )
                nc.scalar.activation(
                    rhs[:, ms:FD], pt[:, ms:FD], AF.Relu, bias=nthr[:, 0:1]
                )

                # scatter-accumulate into PSUM: [S_M2 | T | R]
                rhs5 = rhs[:].rearrange("p (u t c) -> p u t c", u=4, c=C)
                for j in range(ch):
                    first = g == 0 and j == 0
                    last = g == ng - 1 and j == ch - 1
                    nc.tensor.matmul(
                        acc[:, :],
                        rhs5[:, 3, j, :],
                        rhs5[:, 0:3, j, :],
                        start=first,
                        stop=last,
                    )

            da = min(4, ng)  # head runs `da` groups ahead of tailA
            db = min(da + 3, ng)  # tailB two more groups behind (GPS slack)
            for g in range(ng):
                head(g)
                if g >= da:
                    tailA(g - da)
                if g >= db:
                    tailB(g - db)
            for g in range(ng - da, ng):
                tailA(g)
            for g in range(ng - db, ng):
                tailB(g)

            # epilogue: sum of log-partition-functions, dump accumulators
            nc.scalar.activation(ln_t[:], s_all[:], AF.Ln)
            nc.vector.reduce_sum(
                out=out_sb[:, 6 * C : 6 * C + 1], in_=ln_t[:], axis=AX.X, op=OP.add
            )
            nc.vector.tensor_copy(out_sb[:, 0 : 3 * C], acc[:, :])
            nc.vector.memset(out_sb[:, 3 * C : 6 * C], 0.0)
            nc.sync.dma_start(out_ext[:, :], out_sb[:])

    _strip_redundant_dma_lane_waits(nc)
    return nc


def _strip_redundant_dma_lane_waits(nc):
    """Every TPB instruction encoding holds exactly ONE sync-wait slot; walrus
    raises "Too many sync wait commands" on the rest. Legalize every
    multi-wait instruction: keep ONE wait embedded, hoist the rest into
    standalone InstEventSemaphore waits on the same queue immediately before
    the instruction.

    For DMAs the EMBEDDED wait must be the DMA-lane predecessor wait when one
    exists: it enforces in-order completion within the lane, which the
    cumulative semaphore thresholds consumers wait on REQUIRE for soundness
    (out-of-order completion would satisfy a threshold before the data
    landed). Engine waits are hoisted onto the issuing sequencer queue, which
    executes them before pushing the DMA to the ring."""
    f = nc.m.functions[0]
    for blk in list(f.blocks):
        insts = list(blk.instructions)
        new_insts = []
        changed = False
        for inst in insts:
            si = inst.sync_info
            waits = list(si.on_wait) if (si and si.on_wait) else []
            if len(waits) > 1:
                changed = True
                if type(inst).__name__ == "InstDMACopy":
                    lane = [
                        w for w in waits if w.ant_name.startswith(("DMAHW", "DMASW"))
                    ]
                    eng = [
                        w
                        for w in waits
                        if not w.ant_name.startswith(("DMAHW", "DMASW"))
                    ]
                    assert len(lane) <= 1, f"{inst.name}: {len(lane)} lane waits"
                    keep = lane if lane else eng[-1:]
                    extra = eng if lane else eng[:-1]
                else:
                    keep = waits[-1:]
                    extra = waits[:-1]
                for k, w in enumerate(extra):
                    es = mybir.InstEventSemaphore(
                        name=f"{inst.name}-wsplit{k}",
                        engine=inst.engine,
                        ins=[],
                        outs=[],
                        sync_info=mybir.SyncInfo(on_wait=[w], on_update=[]),
                    )
                    nc.register_instruction(es)
                    new_insts.append(es)
                si.on_wait = keep
            new_insts.append(inst)
        if changed:
            blk.instructions = new_insts


def _shard_inputs(outputs: np.ndarray, labels: np.ndarray, rows: int, group_rows: int):
    """Build per-core in_maps. Row mapping inside a core/group: row = g*G + p*ch + t."""
    import ml_dtypes

    bf16 = ml_dtypes.bfloat16
    ch = group_rows // C
    ng = rows // group_rows
    in_maps = []
    n_cores = outputs.shape[0] // rows
    cls = np.arange(C, dtype=np.int32)
    for i in range(n_cores):
        lab_i = labels[i * rows : (i + 1) * rows].astype(np.int32)
        labT = lab_i.reshape(ng, C, ch).transpose(1, 0, 2)  # [C, ng, ch]
        oh = labT[:, :, :, None] == cls[None, None, None, :]  # [C, ng, ch, C]
        xb = (
            outputs[i * rows : (i + 1) * rows]
            .astype(bf16)
            .reshape(ng, C, ch, C)
            .transpose(1, 0, 2, 3)
        )  # [C, ng, ch, C]
        xoh = np.stack([xb, oh.astype(bf16)], axis=2)  # [C, ng, 2, ch, C]
        in_maps.append({"xoh": np.ascontiguousarray(xoh.reshape(C, ng * 2 * group_rows))})
    return in_maps


def combine_outputs(core_outs, lnz_extra=None, confusion_weights=None, B=None):
    """Host-side reduction of per-core [128, 769] partials -> scalar loss."""
    S_M2 = np.zeros((C, C), np.float64)
    T = np.zeros((C, C), np.float64)
    R = np.zeros((C, C), np.float64)
    lnz_sum = 0.0
    for o in core_outs:
        o = np.asarray(o, np.float64)
        for base in (0, 3 * C):
            S_M2 += o[:, base : base + C]
            T += o[:, base + C : base + 2 * C]
            R += o[:, base + 2 * C : base + 3 * C]
        lnz_sum += o[:, 6 * C].sum()
    ce_sum = lnz_sum - np.trace(R)
    base_loss = ce_sum / B

    W = np.asarray(confusion_weights, np.float64)
    wmask = W > WEIGHT_THRESH
    G0 = np.where(wmask, W, 0.0)
    np.fill_diagonal(G0, 0.0)
    H0 = wmask.astype(np.float64)
    np.fill_diagonal(H0, 0.0)

    S = S_M2 + PROB_THRESH * T
    pen_sum = float((G0 * S).sum())
    count = float(np.rint((H0 * T).sum()))
    penalty = pen_sum / max(count, 1.0) if count > 0 else 0.0
    return np.float32(base_loss + CONF_PEN * penalty)


_CACHE = {}


def _get_nc(rows: int, group_rows: int):
    key = (rows, group_rows)
    if key not in _CACHE:
        _CACHE[key] = build_bass(rows, group_rows)
    return _CACHE[key]


def kernel(outputs: np.ndarray, labels: np.ndarray, confusion_weights: np.ndarray, **kw):
    outputs = np.asarray(outputs, np.float32)
    labels = np.asarray(labels)
    B = outputs.shape[0]
    rows = B // N_CORES
    group_rows = GROUP_ROWS
    nc = _get_nc(rows, group_rows)
    in_maps = _shard_inputs(outputs, labels, rows, group_rows)
    res = run_bass_kernel_spmd(nc, in_maps, core_ids=list(range(N_CORES)))
    core_outs = [r["out"] for r in res.results]
    return combine_outputs(core_outs, confusion_weights=confusion_weights, B=B)


if __name__ == "__main__":
    # smoke test on random data (host-side check only builds the graph)
    nc = build_bass(8192, GROUP_ROWS)
    print("built ok:", nc)


# revision 46
# speedup vs baseline: 1.1906x; 1.0112x over previous
"""Trainium2 Bass kernel for AdaptiveFocusedLoss, data-parallel over 8 NeuronCores.

Math (matches the jax reference exactly, up to float rounding):
  logp = log_softmax(outputs); base = -mean(logp[i, l_i])
  probs = softmax(outputs); w = W[l_i]
  mask = (c != l_i) & (w > 1) & (p > 0.2)
  penalty = sum(w*p*mask) / max(count,1) if count>0 else 0
  loss = base + 0.5 * penalty

Device-side pipeline (per core, rows sharded; group layout [p, t, c] with
t = chunk index (ch per group), c innermost so matmul chunks are contiguous):
  e = exp(x)            (ACT, bf16; x = 5*randn bounded ~±30, safe without max-sub)
  s[p,t] = sum_c e      (DVE: one strided f32 tensor_reduce per group)
  r = 1/s               (DVE reciprocal, f32)
  p = e*r               (GPSIMD tensor_tensor with a 3D stride-0 broadcast AP
                         for r; T_P_DVE chunks can be peeled onto DVE)
  A  = [p > 0.2]        (DVE tensor_scalar is_gt, 4x mode) -> rhs region 1
  M2 = max(p-0.2, 0)    (ACT Relu with bias -0.2; T_M2_DVE chunks can be
                         peeled onto DVE dual-op ts) -> rhs region 0
  region 2 = x (bf16, straight from DMA); region 3 = onehot (bf16, from DMA)
  PSUM accumulates over all 128-row chunks:
     S_M2 += O^T @ M2 ; T += O^T @ A ; R += O^T @ x
  epilogue: lnz_sum[p] = sum_t ln(s_all[p,t])
The software pipeline runs three stages (head: DMA+exp; tailA: rowsum+recip+
p-mult; tailB: mask ops+matmuls) at offsets g, g-4, g-7 so the long GPSIMD
multiply of one group overlaps DVE/ACT/PE work of neighboring groups. Buffer
pool depths (ebuf 6 / pbuf 10 / rhsbuf 8) are sized to the producer-consumer
distances; shrinking them reintroduces WAR stalls (measured +10-20% wall).
Host side:
  ce_sum  = sum(lnz) - trace(R)            (trace(R) = sum_i x[i, l_i])
  pen_sum = <G0, S_M2 + 0.2*T>,  count = <H0, T>
  where G0 = W*(W>1) diag-zeroed, H0 = (W>1) diag-zeroed  (c != l mask == zero diag)
"""

import numpy as np

try:
    from concourse import bass, mybir, tile
    from concourse.bass_utils import run_bass_kernel_spmd
except ImportError:  # pragma: no cover
    import sys

    sys.path.insert(0, "/opt/trn_rl_repo")
    from concourse import bass, mybir, tile
    from concourse.bass_utils import run_bass_kernel_spmd

F32 = mybir.dt.float32
BF16 = mybir.dt.bfloat16
AF = mybir.ActivationFunctionType
OP = mybir.AluOpType
AX = mybir.AxisListType

N_CORES = 8
C = 128  # num classes
B_FULL = 524288
PROB_THRESH = 0.2
CONF_PEN = 0.5
WEIGHT_THRESH = 1.0

GROUP_ROWS = 2048  # rows per group (ch = 16 chunks)

# Engine-balance splits along the chunk (t) axis, out of ch chunks/group:
# p = e*r: chunks [0, T_P_DVE) on DVE (3D broadcast tensor_tensor), rest GPSIMD.
# M2:      chunks [0, T_M2_DVE) on DVE (dual-op ts, immediate scalars), rest ACT.
# NOTE: tensor_scalar with an AP scalar (TensorScalarPtr) measured ~28ns/elem
# on HW — never use it for bulk work; immediate-scalar ts is 4x-fast.
T_P_DVE = 0
T_M2_DVE = 0


def build_bass(rows: int, group_rows: int = GROUP_ROWS) -> "bass.Bass":
    """One NeuronCore's graph; SPMD across cores with different shards."""
    assert rows % group_rows == 0 and group_rows % C == 0
    ch = group_rows // C  # chunks (of 128 rows) per group
    ng = rows // group_rows  # groups
    nchunk = rows // C  # total 128-row chunks
    FD = group_rows  # free dim of the big tiles

    nc = bass.Bass()
    # xoh[p, g, 0, t, c] = x_bf16[row(g,p,t), c]; xoh[p, g, 1, t, c] = onehot.
    # One DMA per group loads both with 2*ch*C*2 = 8KB contiguous runs per
    # partition (128 big descriptors).
    xoh_ext = nc.declare_dram_parameter("xoh", [C, ng * 2 * FD], BF16, isOutput=False)
    out_ext = nc.declare_dram_parameter("out", [C, 6 * C + 1], F32, isOutput=True)
    xoh_view = xoh_ext[:, :].rearrange("p (g u f) -> p g u f", g=ng, u=2)

    with tile.TileContext(nc) as tc:
        with (
            tc.tile_pool(name="const", bufs=1) as constp,
            tc.tile_pool(name="ebuf", bufs=6) as ep,
            tc.tile_pool(name="pbuf", bufs=10) as pp,
            tc.tile_pool(name="rhsbuf", bufs=8) as rhsp,
            tc.tile_pool(name="small", bufs=8) as smallp,
            tc.tile_pool(name="psum", bufs=1, space="PSUM") as psp,
        ):
            s_all = constp.tile([C, nchunk], F32)
            ln_t = constp.tile([C, nchunk], F32)
            out_sb = constp.tile([C, 6 * C + 1], F32)
            nthr = constp.tile([C, 1], F32)  # -PROB_THRESH bias for ACT Relu
            acc = psp.tile([C, 3 * C], F32)
            nc.vector.memset(nthr[:], -PROB_THRESH)

            state = {}

            def head(g):
                """DMA + exp for group g (emitted ahead of tail).
                rhs regions: [M2(FD) | A(FD) | X(FD) | OH(FD)]."""
                et = ep.tile([C, FD], BF16, tag="et")
                rhs = rhsp.tile([C, 4 * FD], BF16, tag="rhs")
                rhs4 = rhs[:].rearrange("p (u f) -> p u f", u=4)
                nc.sync.dma_start(rhs4[:, 2:4, :], xoh_view[:, g, :, :])
                nc.scalar.activation(et[:], rhs[:, 2 * FD : 3 * FD], AF.Exp)
                state[g] = (et, rhs)

            def tailA(g):
                """Rowsum + recip + DVE share of p, and kick off the GPSIMD
                p-multiply. Emitted one group ahead of tailB so DVE/ACT work
                of group g overlaps GPSIMD's long multiply of group g."""
                et, rhs = state.pop(g)
                pt = pp.tile([C, FD], BF16, tag="pt")
                rt = smallp.tile([C, ch], F32, tag="rt")

                # rowsum: one strided f32 reduce per group (fewer DVE ops beats
                # the bf16 halving tree once per-op stall overhead is counted)
                e3 = et[:].rearrange("p (t c) -> p t c", t=ch)
                ssl = s_all[:, g * ch : (g + 1) * ch]
                nc.vector.reduce_sum(out=ssl, in_=e3[:], axis=AX.X)
                nc.vector.reciprocal(rt[:], ssl)

                # p = e * r (DVE chunk share + GPSIMD broadcast mult)
                pt3 = pt[:].rearrange("p (t c) -> p t c", t=ch)
                cs = T_P_DVE
                rtb = rt[:].rearrange("p (t x) -> p t x", x=1)
                with nc.allow_low_precision(reason="bf16 p"):
                    if cs > 0:
                        nc.vector.tensor_tensor(
                            pt3[:, :cs, :],
                            e3[:, :cs, :],
                            rtb[:, :cs, :].to_broadcast([C, cs, C]),
                            OP.mult,
                        )
                    nc.gpsimd.tensor_tensor(
                        pt3[:, cs:, :],
                        e3[:, cs:, :],
                        rtb[:, cs:, :].to_broadcast([C, ch - cs, C]),
                        OP.mult,
                    )
                state[("b", g)] = (rhs, pt)

            def tailB(g):
                """Mask ops (need the full p) + matmuls for group g."""
                rhs, pt = state.pop(("b", g))
                # A = [p > 0.2] -> region 1 (DVE tensor_scalar, 4x mode)
                nc.vector.tensor_scalar(
                    rhs[:, FD : 2 * FD], pt[:], PROB_THRESH, None, OP.is_gt
                )
                # M2 = max(p - 0.2, 0) -> region 0 (DVE dual-op ts | ACT Relu)
                ms = T_M2_DVE * C
                if ms > 0:
                    nc.vector.tensor_scalar(
                        rhs[:, 0:ms], pt[:, 0:ms], PROB_THRESH, 0.0, OP.subtract, OP.max
                    )
                nc.scalar.activation(
                    rhs[:, ms:FD], pt[:, ms:FD], AF.Relu, bias=nthr[:, 0:1]
                )

                # scatter-accumulate into PSUM: [S_M2 | T | R]
                rhs5 = rhs[:].rearrange("p (u t c) -> p u t c", u=4, c=C)
                for j in range(ch):
                    first = g == 0 and j == 0
                    last = g == ng - 1 and j == ch - 1
                    nc.tensor.matmul(
                        acc[:, :],
                        rhs5[:, 3, j, :],
                        rhs5[:, 0:3, j, :],
                        start=first,
                        stop=last,
                    )

            da = min(4, ng)  # head runs `da` groups ahead of tailA
            db = min(da + 3, ng)  # tailB three more groups behind (GPS slack)
            # tailB first: its is_gt/Relu gate the matmuls (whose completion
            # recycles rhs buffers for head), so they go ahead of the newer
            # groups' EXP/reduce in the ACT/DVE queues.
            for g in range(ng):
                if g >= db:
                    tailB(g - db)
                head(g)
                if g >= da:
                    tailA(g - da)
            for g in range(ng - da, ng):
                tailA(g)
            for g in range(ng - db, ng):
                tailB(g)

            # epilogue: sum of log-partition-functions, dump accumulators
            nc.scalar.activation(ln_t[:], s_all[:], AF.Ln)
            nc.vector.reduce_sum(
                out=out_sb[:, 6 * C : 6 * C + 1], in_=ln_t[:], axis=AX.X, op=OP.add
            )
            nc.vector.tensor_copy(out_sb[:, 0 : 3 * C], acc[:, :])
            nc.vector.memset(out_sb[:, 3 * C : 6 * C], 0.0)
            nc.sync.dma_start(out_ext[:, :], out_sb[:])

    _strip_redundant_dma_lane_waits(nc)
    return nc


def _strip_redundant_dma_lane_waits(nc):
    """Every TPB instruction encoding holds exactly ONE sync-wait slot; walrus
    raises "Too many sync wait commands" on the rest. Legalize every
    multi-wait instruction: keep ONE wait embedded, hoist the rest into
    standalone InstEventSemaphore waits on the same queue immediately before
    the instruction.

    For DMAs the EMBEDDED wait must be the DMA-lane predecessor wait when one
    exists: it enforces in-order completion within the lane, which the
    cumulative semaphore thresholds consumers wait on REQUIRE for soundness
    (out-of-order completion would satisfy a threshold before the data
    landed). Engine waits are hoisted onto the issuing sequencer queue, which
    executes them before pushing the DMA to the ring."""
    f = nc.m.functions[0]
    for blk in list(f.blocks):
        insts = list(blk.instructions)
        new_insts = []
        changed = False
        for inst in insts:
            si = inst.sync_info
            waits = list(si.on_wait) if (si and si.on_wait) else []
            if len(waits) > 1:
                changed = True
                if type(inst).__name__ == "InstDMACopy":
                    lane = [
                        w for w in waits if w.ant_name.startswith(("DMAHW", "DMASW"))
                    ]
                    eng = [
                        w
                        for w in waits
                        if not w.ant_name.startswith(("DMAHW", "DMASW"))
                    ]
                    assert len(lane) <= 1, f"{inst.name}: {len(lane)} lane waits"
                    keep = lane if lane else eng[-1:]
                    extra = eng if lane else eng[:-1]
                else:
                    keep = waits[-1:]
                    extra = waits[:-1]
                for k, w in enumerate(extra):
                    es = mybir.InstEventSemaphore(
                        name=f"{inst.name}-wsplit{k}",
                        engine=inst.engine,
                        ins=[],
                        outs=[],
                        sync_info=mybir.SyncInfo(on_wait=[w], on_update=[]),
                    )
                    nc.register_instruction(es)
                    new_insts.append(es)
                si.on_wait = keep
            new_insts.append(inst)
        if changed:
            blk.instructions = new_insts


def _shard_inputs(outputs: np.ndarray, labels: np.ndarray, rows: int, group_rows: int):
    """Build per-core in_maps. Row mapping inside a core/group: row = g*G + p*ch + t."""
    import ml_dtypes

    bf16 = ml_dtypes.bfloat16
    ch = group_rows // C
    ng = rows // group_rows
    in_maps = []
    n_cores = outputs.shape[0] // rows
    cls = np.arange(C, dtype=np.int32)
    for i in range(n_cores):
        lab_i = labels[i * rows : (i + 1) * rows].astype(np.int32)
        labT = lab_i.reshape(ng, C, ch).transpose(1, 0, 2)  # [C, ng, ch]
        oh = labT[:, :, :, None] == cls[None, None, None, :]  # [C, ng, ch, C]
        xb = (
            outputs[i * rows : (i + 1) * rows]
            .astype(bf16)
            .reshape(ng, C, ch, C)
            .transpose(1, 0, 2, 3)
        )  # [C, ng, ch, C]
        xoh = np.stack([xb, oh.astype(bf16)], axis=2)  # [C, ng, 2, ch, C]
        in_maps.append({"xoh": np.ascontiguousarray(xoh.reshape(C, ng * 2 * group_rows))})
    return in_maps


def combine_outputs(core_outs, lnz_extra=None, confusion_weights=None, B=None):
    """Host-side reduction of per-core [128, 769] partials -> scalar loss."""
    S_M2 = np.zeros((C, C), np.float64)
    T = np.zeros((C, C), np.float64)
    R = np.zeros((C, C), np.float64)
    lnz_sum = 0.0
    for o in core_outs:
        o = np.asarray(o, np.float64)
        for base in (0, 3 * C):
            S_M2 += o[:, base : base + C]
            T += o[:, base + C : base + 2 * C]
            R += o[:, base + 2 * C : base + 3 * C]
        lnz_sum += o[:, 6 * C].sum()
    ce_sum = lnz_sum - np.trace(R)
    base_loss = ce_sum / B

    W = np.asarray(confusion_weights, np.float64)
    wmask = W > WEIGHT_THRESH
    G0 = np.where(wmask, W, 0.0)
    np.fill_diagonal(G0, 0.0)
    H0 = wmask.astype(np.float64)
    np.fill_diagonal(H0, 0.0)

    S = S_M2 + PROB_THRESH * T
    pen_sum = float((G0 * S).sum())
    count = float(np.rint((H0 * T).sum()))
    penalty = pen_sum / max(count, 1.0) if count > 0 else 0.0
    return np.float32(base_loss + CONF_PEN * penalty)


_CACHE = {}


def _get_nc(rows: int, group_rows: int):
    key = (rows, group_rows)
    if key not in _CACHE:
        _CACHE[key] = build_bass(rows, group_rows)
    return _CACHE[key]


def kernel(outputs: np.ndarray, labels: np.ndarray, confusion_weights: np.ndarray, **kw):
    outputs = np.asarray(outputs, np.float32)
    labels = np.asarray(labels)
    B = outputs.shape[0]
    rows = B // N_CORES
    group_rows = GROUP_ROWS
    nc = _get_nc(rows, group_rows)
    in_maps = _shard_inputs(outputs, labels, rows, group_rows)
    res = run_bass_kernel_spmd(nc, in_maps, core_ids=list(range(N_CORES)))
    core_outs = [r["out"] for r in res.results]
    return combine_outputs(core_outs, confusion_weights=confusion_weights, B=B)


if __name__ == "__main__":
    # smoke test on random data (host-side check only builds the graph)
    nc = build_bass(8192, GROUP_ROWS)
    print("built ok:", nc)
